# revision 4
# baseline (speedup 1.0000x reference)
"""Bidirectional Chamfer loss on 8 Trainium2 NeuronCores — v2.

Same math/packing/host-combine as v1; reworked device inner loop:
  - super-tiles of 4096 pred-cols (4 chunks nonfilt / 8 chunks filt)
  - ACT copies 2048-wide (4 PSUM banks per copy, 2 copies per st)
  - pred-side: ONE full-width tt(min) into a [128,4096] multi-slot
    accumulator per st (slots merged once per batch)
  - gt-side: fused 4D-AP fold1..fold4 (all 2x bf16) then one strided
    group tensor_reduce per 4 super-tiles
"""

import numpy as np

B = 4
NF = 4096
NN = 8192
NCORES = 8
RF = NF // NCORES   # 512 pred rows per core (filtered)
RN = NN // NCORES   # 1024 pred rows per core (nonfiltered)
K24 = 24            # contraction rows of the split-bf16 matmul
ST = 8192           # super-tile width (pred-col elements per st)

# output column layout
N_M1 = B * 2 * ST                           # raw pred accumulators (bf16)
STQ = ST // 8                               # filtered fold3 residue cols per st
STQN = ST // 4                              # nonfiltered fold2 residue cols per st
N_M2 = B * 2 * STQ + B * 8 * STQN           # gt residues (bf16): 8192 + 65536

_CACHE = {}


def build_nc():
    from contextlib import ExitStack

    import concourse.mybir as mybir
    import concourse.tile as tile
    from concourse import bacc

    f32 = mybir.dt.float32
    bf16 = mybir.dt.bfloat16
    Alu = mybir.AluOpType

    nc = bacc.Bacc("TRN2", target_bir_lowering=False, debug=False)

    Pf = nc.dram_tensor("pf", [B, K24, RF], bf16, kind="ExternalInput").ap()
    Gf = nc.dram_tensor("gf", [B, K24, NF], bf16, kind="ExternalInput").ap()
    Pn = nc.dram_tensor("pn", [B, K24, RN], bf16, kind="ExternalInput").ap()
    Gn = nc.dram_tensor("gn", [B, K24, NN], bf16, kind="ExternalInput").ap()
    O1 = nc.dram_tensor("m1", [128, N_M1], bf16, kind="ExternalOutput").ap()
    O2 = nc.dram_tensor("m2", [128, N_M2], bf16, kind="ExternalOutput").ap()

    with tile.TileContext(nc) as tc, ExitStack() as ctx:
        gpool = ctx.enter_context(tc.tile_pool(name="gt", bufs=2))
        ppool = ctx.enter_context(tc.tile_pool(name="pred", bufs=2))
        cpool = ctx.enter_context(tc.tile_pool(name="copy", bufs=3))
        fpool = ctx.enter_context(tc.tile_pool(name="fold", bufs=2))
        apool = ctx.enter_context(tc.tile_pool(name="pacc", bufs=2))
        out_pool = ctx.enter_context(tc.tile_pool(name="outs", bufs=1))
        psum_pool = ctx.enter_context(tc.tile_pool(name="psum", bufs=2, space="PSUM"))

        cfgs = ((Pf, Gf, NF, RF), (Pn, Gn, NN, RN))
        m1base = (0, B * ST)
        m2base = (0, B * 2 * STQ)
        # interleave filtered (ACT-paced) and nonfiltered (DVE-paced) batches
        jobs = []
        for b in range(B):
            jobs.append((0, b))
            jobs.append((1, b))
        for ji, (ci, b) in enumerate(jobs):
            last_job = ji == len(jobs) - 1
            Pt, Gt, Npts, Rrows = cfgs[ci]
            n_jt = Npts // 128          # gt chunks per batch
            SG = ST // Rrows            # chunks per super-tile (16 filt / 8 nonfilt)
            n_st = n_jt // SG           # super-tiles per batch (2 / 8)
            stq = STQ if ci == 0 else STQN
            m1col = m1base[ci] + b * ST
            m2col = m2base[ci] + b * n_st * stq
            if True:
                sG = gpool.tile([K24, Npts], bf16, tag=f"gt{ci}")
                for s in range(Npts // 2048):
                    nc.sync.dma_start(
                        sG[:, s * 2048 : (s + 1) * 2048],
                        Gt[b][:, s * 2048 : (s + 1) * 2048],
                    )
                sP = ppool.tile([K24, Rrows], bf16, tag=f"pred{ci}")
                nc.sync.dma_start(sP[:], Pt[b])
                pa = apool.tile([128, ST], bf16, tag="pacc")

                for st in range(n_st):
                    # st 0: ACT copies land directly in the accumulator
                    cp = pa if st == 0 else cpool.tile([128, ST], bf16, tag="copy")
                    for half in range(4):
                        ps = psum_pool.tile([128, 2048], f32, tag="ps")
                        for m in range(4):
                            gcol = half * 2048 + m * 512
                            jt = st * SG + gcol // Rrows
                            h = (gcol % Rrows) // 512
                            nc.tensor.matmul(
                                ps[:, m * 512 : (m + 1) * 512],
                                lhsT=sG[:, jt * 128 : (jt + 1) * 128],
                                rhs=sP[:, h * 512 : (h + 1) * 512],
                                start=True,
                                stop=True,
                            )
                        nc.scalar.copy(cp[:, half * 2048 : (half + 1) * 2048], ps[:])
                    # pred-side: one full-width accumulate (slots merged on host)
                    if st > 0:
                        if last_job and st == n_st - 1:
                            # split final accumulate so the pa DMA can overlap
                            for hb in range(2):
                                sl = slice(hb * 4096, (hb + 1) * 4096)
                                nc.vector.tensor_tensor(
                                    out=pa[:, sl], in0=cp[:, sl], in1=pa[:, sl], op=Alu.min
                                )
                                nc.sync.dma_start(
                                    O1[:, m1col + hb * 4096 : m1col + (hb + 1) * 4096],
                                    pa[:, sl],
                                )
                        else:
                            nc.vector.tensor_tensor(out=pa[:], in0=cp[:], in1=pa[:], op=Alu.min)
                    # gt-side fold chain (within-chunk halves, c-major 4D APs)
                    w = Rrows // 2
                    cur = cp
                    levels = ((1, 4096), (2, 2048), (3, 1024)) if ci == 0 else ((1, 4096), (2, 2048))
                    for lvl, width in levels:
                        nxt = fpool.tile([128, width], bf16, tag=f"t{lvl}")
                        if lvl == 1 and st == 0:
                            # split fold1 so DVE starts after the first copy
                            for hb in range(4):
                                nc.vector.tensor_tensor(
                                    out=nxt[:, hb * 1024 : (hb + 1) * 1024].rearrange(
                                        "p (c o k) -> p c o k", c=SG // 4, o=1, k=w
                                    ),
                                    in0=cur[:, hb * 2048 : (hb + 1) * 2048].rearrange(
                                        "p (c h k) -> p c h k", c=SG // 4, h=2, k=w
                                    )[:, :, 0:1, :],
                                    in1=cur[:, hb * 2048 : (hb + 1) * 2048].rearrange(
                                        "p (c h k) -> p c h k", c=SG // 4, h=2, k=w
                                    )[:, :, 1:2, :],
                                    op=Alu.min,
                                )
                        else:
                            nc.vector.tensor_tensor(
                                out=nxt[:].rearrange("p (c o k) -> p c o k", c=SG, o=1, k=w),
                                in0=cur[:].rearrange("p (c h k) -> p c h k", c=SG, h=2, k=w)[:, :, 0:1, :],
                                in1=cur[:].rearrange("p (c h k) -> p c h k", c=SG, h=2, k=w)[:, :, 1:2, :],
                                op=Alu.min,
                            )
                        cur = nxt
                        w //= 2
                    # ship gt fold residues; host finishes the gt-side mins
                    nc.sync.dma_start(
                        O2[:, m2col + st * stq : m2col + (st + 1) * stq], cur[:]
                    )
                # ship the raw accumulator; host merges slots + partitions
                if not last_job:
                    for hb in range(2):
                        nc.sync.dma_start(
                            O1[:, m1col + hb * 4096 : m1col + (hb + 1) * 4096],
                            pa[:, hb * 4096 : (hb + 1) * 4096],
                        )


    nc.compile()
    return nc


def _split3(x):
    import ml_dtypes

    bf = ml_dtypes.bfloat16
    b0 = x.astype(bf)
    r1 = (x - b0.astype(np.float32)).astype(np.float32)
    b1 = r1.astype(bf)
    r2 = (r1 - b1.astype(np.float32)).astype(np.float32)
    b2 = r2.astype(bf)
    return b0, b1, b2


_PAIRS = ((0, 0), (0, 1), (1, 0), (0, 2), (1, 1), (2, 0))


def pack_inputs(pred_filtered, gt_filtered, pred_nonfiltered, gt_nonfiltered):
    import ml_dtypes

    bf = ml_dtypes.bfloat16

    def mk(p, q):
        p = p.astype(np.float32)
        q = q.astype(np.float32)
        Bn, Np_, _ = p.shape
        Nq = q.shape[1]
        P = np.zeros((Bn, K24, Np_), bf)
        G = np.zeros((Bn, K24, Nq), bf)
        pp = np.sum(p * p, axis=-1, dtype=np.float32)
        qq = np.sum(q * q, axis=-1, dtype=np.float32)
        for c in range(3):
            ws = _split3(-2.0 * p[..., c])
            gs = _split3(q[..., c])
            for t, (gi, wi) in enumerate(_PAIRS):
                G[:, 6 * c + t, :] = gs[gi]
                P[:, 6 * c + t, :] = ws[wi]
        qqs = _split3(qq)
        pps = _split3(pp)
        for t in range(3):
            G[:, 18 + t, :] = qqs[t]
            P[:, 18 + t, :] = np.ones_like(pp, dtype=bf)
            G[:, 21 + t, :] = np.ones_like(qq, dtype=bf)
            P[:, 21 + t, :] = pps[t]
        return P, G

    pf_all, gf = mk(pred_filtered, gt_filtered)
    pn_all, gn = mk(pred_nonfiltered, gt_nonfiltered)
    gf = np.ascontiguousarray(gf)
    gn = np.ascontiguousarray(gn)

    in_maps = []
    for k in range(NCORES):
        in_maps.append(
            {
                "pf": np.ascontiguousarray(pf_all[:, :, k * RF : (k + 1) * RF]),
                "gf": gf,
                "pn": np.ascontiguousarray(pn_all[:, :, k * RN : (k + 1) * RN]),
                "gn": gn,
            }
        )
    return in_maps


def combine_outputs(results):
    cds = {}
    for cfg, (Npts, Rrows, m1off, m2off) in (
        ("f", (NF, RF, 0, 0)),
        ("n", (NN, RN, B * ST, B * 2 * STQ)),  # n residues at fold2 depth
    ):
        n_st = Npts // 128 // (ST // Rrows)
        slots = ST // Rrows
        # raw accumulators: [cores, 128, B, slots, Rrows]; min over partitions+slots
        m1 = np.stack(
            [r["m1"][:, m1off : m1off + B * ST].astype(np.float32) for r in results]
        ).reshape(NCORES, 128, B, slots, Rrows)
        pred_mean = m1.min(axis=(1, 3)).mean(axis=(0, 2))
        # fold residues: [cores, 128, B, n_st, SG, kq]
        # gt point (b, st*SG + c, p) -> min over k (and cores)
        kq = Rrows // 8 if cfg == "f" else Rrows // 4
        stq = (ST // Rrows) * kq
        m2 = np.stack(
            [
                r["m2"][:, m2off : m2off + B * n_st * stq].astype(np.float32)
                for r in results
            ]
        ).reshape(NCORES, 128, B, n_st, slots, kq)
        gt_min = m2.min(axis=(0, 5))          # [128, B, n_st, SG]
        gt_mean = gt_min.mean(axis=(0, 2, 3))  # [B]
        cds[cfg] = (pred_mean + gt_mean).mean()
    return np.float32(0.7 * cds["f"] + 0.3 * cds["n"])


def kernel(pred_filtered, gt_filtered, pred_nonfiltered, gt_nonfiltered):
    from concourse.bass_utils import run_bass_kernel_spmd

    if "nc" not in _CACHE:
        _CACHE["nc"] = build_nc()
    in_maps = pack_inputs(
        pred_filtered, gt_filtered, pred_nonfiltered, gt_nonfiltered
    )
    res = run_bass_kernel_spmd(_CACHE["nc"], in_maps, core_ids=list(range(NCORES)))
    return combine_outputs(res.results)


# revision 5
# speedup vs baseline: 1.0006x; 1.0006x over previous
"""Bidirectional Chamfer loss on 8 Trainium2 NeuronCores.

Math per batch pair (p, q):  D[i,j] = ||p_i||^2 + ||q_j||^2 - 2 p_i.q_j
(split-bf16 K=24 matmul, ~2^-27 error/term); loss = 0.7*mean cd_filt +
0.3*mean cd_nonfilt with cd = mean_i min_j D + mean_j min_i D.

Mapping: pred rows sharded 8 ways, gt replicated; psum[gt_pt, pred_row].
Device inner loop (ACT crossing-bound ~307us, DVE ~95% busy):
  - super-tiles of 8192 pred-cols; 16 N=512 matmuls into 2x [128,2048]
    PSUM tiles per st; four 2048-wide fp32->bf16 ACT copies per st (the
    hard bottleneck: every D element crosses PSUM->SBUF once, 1 elem/cyc)
  - pred-side: one full-width tt(min) per st into a [128,8192] multi-slot
    accumulator (st 0 aliases the ACT copies straight into it); raw
    accumulator shipped, host takes min over slots+partitions
  - gt-side: fused 4D-AP 2x-bf16 fold tree, depth 3 on filtered batches
    (ACT-paced, DVE rides free) but depth 2 on nonfiltered (DVE-paced)
    to shed DVE cycles; residues DMA'd, host finishes the row mins
  - filtered/nonfiltered batches interleaved so ACT-paced and DVE-paced
    phases overlap
"""

import numpy as np

B = 4
NF = 4096
NN = 8192
NCORES = 8
RF = NF // NCORES   # 512 pred rows per core (filtered)
RN = NN // NCORES   # 1024 pred rows per core (nonfiltered)
K24 = 24            # contraction rows of the split-bf16 matmul
ST = 8192           # super-tile width (pred-col elements per st)

# output column layout
N_M1 = B * 2 * ST                           # raw pred accumulators (bf16)
STQ = ST // 8                               # filtered fold3 residue cols per st
STQN = ST // 4                              # nonfiltered fold2 residue cols per st
N_M2 = B * 2 * STQ + B * 8 * STQN           # gt residues (bf16): 8192 + 65536

_CACHE = {}


def build_nc():
    from contextlib import ExitStack

    import concourse.mybir as mybir
    import concourse.tile as tile
    from concourse import bacc

    f32 = mybir.dt.float32
    bf16 = mybir.dt.bfloat16
    Alu = mybir.AluOpType

    nc = bacc.Bacc("TRN2", target_bir_lowering=False, debug=False)

    Pf = nc.dram_tensor("pf", [B, K24, RF], bf16, kind="ExternalInput").ap()
    Gf = nc.dram_tensor("gf", [B, K24, NF], bf16, kind="ExternalInput").ap()
    Pn = nc.dram_tensor("pn", [B, K24, RN], bf16, kind="ExternalInput").ap()
    Gn = nc.dram_tensor("gn", [B, K24, NN], bf16, kind="ExternalInput").ap()
    O1 = nc.dram_tensor("m1", [128, N_M1], bf16, kind="ExternalOutput").ap()
    O2 = nc.dram_tensor("m2", [128, N_M2], bf16, kind="ExternalOutput").ap()

    with tile.TileContext(nc) as tc, ExitStack() as ctx:
        gpool = ctx.enter_context(tc.tile_pool(name="gt", bufs=2))
        ppool = ctx.enter_context(tc.tile_pool(name="pred", bufs=2))
        cpool = ctx.enter_context(tc.tile_pool(name="copy", bufs=3))
        fpool = ctx.enter_context(tc.tile_pool(name="fold", bufs=2))
        apool = ctx.enter_context(tc.tile_pool(name="pacc", bufs=2))
        out_pool = ctx.enter_context(tc.tile_pool(name="outs", bufs=1))
        psum_pool = ctx.enter_context(tc.tile_pool(name="psum", bufs=2, space="PSUM"))

        cfgs = ((Pf, Gf, NF, RF), (Pn, Gn, NN, RN))
        m1base = (0, B * ST)
        m2base = (0, B * 2 * STQ)
        # interleave filtered (ACT-paced) and nonfiltered (DVE-paced) batches
        jobs = []
        for b in range(B):
            jobs.append((0, b))
            jobs.append((1, b))
        for ji, (ci, b) in enumerate(jobs):
            last_job = ji == len(jobs) - 1
            Pt, Gt, Npts, Rrows = cfgs[ci]
            n_jt = Npts // 128          # gt chunks per batch
            SG = ST // Rrows            # chunks per super-tile (16 filt / 8 nonfilt)
            n_st = n_jt // SG           # super-tiles per batch (2 / 8)
            stq = STQ if ci == 0 else STQN
            m1col = m1base[ci] + b * ST
            m2col = m2base[ci] + b * n_st * stq
            if True:
                sG = gpool.tile([K24, Npts], bf16, tag=f"gt{ci}")
                for s in range(Npts // 2048):
                    nc.sync.dma_start(
                        sG[:, s * 2048 : (s + 1) * 2048],
                        Gt[b][:, s * 2048 : (s + 1) * 2048],
                    )
                sP = ppool.tile([K24, Rrows], bf16, tag=f"pred{ci}")
                nc.sync.dma_start(sP[:], Pt[b])
                pa = apool.tile([128, ST], bf16, tag="pacc")

                for st in range(n_st):
                    # st 0: ACT copies land directly in the accumulator
                    cp = pa if st == 0 else cpool.tile([128, ST], bf16, tag="copy")
                    for half in range(4):
                        ps = psum_pool.tile([128, 2048], f32, tag="ps")
                        for m in range(4):
                            gcol = half * 2048 + m * 512
                            jt = st * SG + gcol // Rrows
                            h = (gcol % Rrows) // 512
                            nc.tensor.matmul(
                                ps[:, m * 512 : (m + 1) * 512],
                                lhsT=sG[:, jt * 128 : (jt + 1) * 128],
                                rhs=sP[:, h * 512 : (h + 1) * 512],
                                start=True,
                                stop=True,
                            )
                        nc.scalar.copy(cp[:, half * 2048 : (half + 1) * 2048], ps[:])
                    # pred-side: one full-width accumulate (slots merged on host)
                    if st > 0:
                        if last_job and st == n_st - 1:
                            # split final accumulate so the pa DMA can overlap
                            for hb in range(2):
                                sl = slice(hb * 4096, (hb + 1) * 4096)
                                nc.vector.tensor_tensor(
                                    out=pa[:, sl], in0=cp[:, sl], in1=pa[:, sl], op=Alu.min
                                )
                                nc.sync.dma_start(
                                    O1[:, m1col + hb * 4096 : m1col + (hb + 1) * 4096],
                                    pa[:, sl],
                                )
                        else:
                            nc.vector.tensor_tensor(out=pa[:], in0=cp[:], in1=pa[:], op=Alu.min)
                    # gt-side fold chain (within-chunk halves, c-major 4D APs)
                    w = Rrows // 2
                    cur = cp
                    levels = ((1, 4096), (2, 2048), (3, 1024)) if ci == 0 else ((1, 4096), (2, 2048))
                    for lvl, width in levels:
                        nxt = fpool.tile([128, width], bf16, tag=f"t{lvl}")
                        if lvl == 1 and st == 0:
                            # split fold1 so DVE starts after the first copy
                            for hb in range(4):
                                nc.vector.tensor_tensor(
                                    out=nxt[:, hb * 1024 : (hb + 1) * 1024].rearrange(
                                        "p (c o k) -> p c o k", c=SG // 4, o=1, k=w
                                    ),
                                    in0=cur[:, hb * 2048 : (hb + 1) * 2048].rearrange(
                                        "p (c h k) -> p c h k", c=SG // 4, h=2, k=w
                                    )[:, :, 0:1, :],
                                    in1=cur[:, hb * 2048 : (hb + 1) * 2048].rearrange(
                                        "p (c h k) -> p c h k", c=SG // 4, h=2, k=w
                                    )[:, :, 1:2, :],
                                    op=Alu.min,
                                )
                        else:
                            nc.vector.tensor_tensor(
                                out=nxt[:].rearrange("p (c o k) -> p c o k", c=SG, o=1, k=w),
                                in0=cur[:].rearrange("p (c h k) -> p c h k", c=SG, h=2, k=w)[:, :, 0:1, :],
                                in1=cur[:].rearrange("p (c h k) -> p c h k", c=SG, h=2, k=w)[:, :, 1:2, :],
                                op=Alu.min,
                            )
                        cur = nxt
                        w //= 2
                    # ship gt fold residues; host finishes the gt-side mins
                    nc.sync.dma_start(
                        O2[:, m2col + st * stq : m2col + (st + 1) * stq], cur[:]
                    )
                # ship the raw accumulator; host merges slots + partitions
                if not last_job:
                    for hb in range(2):
                        nc.sync.dma_start(
                            O1[:, m1col + hb * 4096 : m1col + (hb + 1) * 4096],
                            pa[:, hb * 4096 : (hb + 1) * 4096],
                        )


    nc.compile()
    return nc


def _split3(x):
    import ml_dtypes

    bf = ml_dtypes.bfloat16
    b0 = x.astype(bf)
    r1 = (x - b0.astype(np.float32)).astype(np.float32)
    b1 = r1.astype(bf)
    r2 = (r1 - b1.astype(np.float32)).astype(np.float32)
    b2 = r2.astype(bf)
    return b0, b1, b2


_PAIRS = ((0, 0), (0, 1), (1, 0), (0, 2), (1, 1), (2, 0))


def pack_inputs(pred_filtered, gt_filtered, pred_nonfiltered, gt_nonfiltered):
    import ml_dtypes

    bf = ml_dtypes.bfloat16

    def mk(p, q):
        p = p.astype(np.float32)
        q = q.astype(np.float32)
        Bn, Np_, _ = p.shape
        Nq = q.shape[1]
        P = np.zeros((Bn, K24, Np_), bf)
        G = np.zeros((Bn, K24, Nq), bf)
        pp = np.sum(p * p, axis=-1, dtype=np.float32)
        qq = np.sum(q * q, axis=-1, dtype=np.float32)
        for c in range(3):
            ws = _split3(-2.0 * p[..., c])
            gs = _split3(q[..., c])
            for t, (gi, wi) in enumerate(_PAIRS):
                G[:, 6 * c + t, :] = gs[gi]
                P[:, 6 * c + t, :] = ws[wi]
        qqs = _split3(qq)
        pps = _split3(pp)
        for t in range(3):
            G[:, 18 + t, :] = qqs[t]
            P[:, 18 + t, :] = np.ones_like(pp, dtype=bf)
            G[:, 21 + t, :] = np.ones_like(qq, dtype=bf)
            P[:, 21 + t, :] = pps[t]
        return P, G

    pf_all, gf = mk(pred_filtered, gt_filtered)
    pn_all, gn = mk(pred_nonfiltered, gt_nonfiltered)
    gf = np.ascontiguousarray(gf)
    gn = np.ascontiguousarray(gn)

    in_maps = []
    for k in range(NCORES):
        in_maps.append(
            {
                "pf": np.ascontiguousarray(pf_all[:, :, k * RF : (k + 1) * RF]),
                "gf": gf,
                "pn": np.ascontiguousarray(pn_all[:, :, k * RN : (k + 1) * RN]),
                "gn": gn,
            }
        )
    return in_maps


def combine_outputs(results):
    cds = {}
    for cfg, (Npts, Rrows, m1off, m2off) in (
        ("f", (NF, RF, 0, 0)),
        ("n", (NN, RN, B * ST, B * 2 * STQ)),  # n residues at fold2 depth
    ):
        n_st = Npts // 128 // (ST // Rrows)
        slots = ST // Rrows
        # raw accumulators: [cores, 128, B, slots, Rrows]; min over partitions+slots
        m1 = np.stack(
            [r["m1"][:, m1off : m1off + B * ST].astype(np.float32) for r in results]
        ).reshape(NCORES, 128, B, slots, Rrows)
        pred_mean = m1.min(axis=(1, 3)).mean(axis=(0, 2))
        # fold residues: [cores, 128, B, n_st, SG, kq]
        # gt point (b, st*SG + c, p) -> min over k (and cores)
        kq = Rrows // 8 if cfg == "f" else Rrows // 4
        stq = (ST // Rrows) * kq
        m2 = np.stack(
            [
                r["m2"][:, m2off : m2off + B * n_st * stq].astype(np.float32)
                for r in results
            ]
        ).reshape(NCORES, 128, B, n_st, slots, kq)
        gt_min = m2.min(axis=(0, 5))          # [128, B, n_st, SG]
        gt_mean = gt_min.mean(axis=(0, 2, 3))  # [B]
        cds[cfg] = (pred_mean + gt_mean).mean()
    return np.float32(0.7 * cds["f"] + 0.3 * cds["n"])


def kernel(pred_filtered, gt_filtered, pred_nonfiltered, gt_nonfiltered):
    from concourse.bass_utils import run_bass_kernel_spmd

    if "nc" not in _CACHE:
        _CACHE["nc"] = build_nc()
    in_maps = pack_inputs(
        pred_filtered, gt_filtered, pred_nonfiltered, gt_nonfiltered
    )
    res = run_bass_kernel_spmd(_CACHE["nc"], in_maps, core_ids=list(range(NCORES)))
    return combine_outputs(res.results)
